# revision 1
# baseline (speedup 1.0000x reference)
"""Supervised-contrastive loss (balanced softmax variant) on 8 Trainium2 cores.

Data-parallel over the 8192 feature rows: each core computes the full
[1024, 9192] logits block for its rows in a fused streaming fashion
(matmul -> exp -> masked reductions, nothing round-trips to HBM), producing
per-row loss terms; host averages the 8 partials.

Math (per row i, shift s=10 which is ~the row max since rows are unit norm;
the loss is exactly shift-invariant):
    z_ij   = 10 * f_i . A_j             A = [features; centers]
    E'_ij  = exp(z_ij - 10 + ln a_j)    a_j = 1/cls_count[t_all_j]  (via a
                                        K=1 bias-row matmul into PSUM)
    S_a_i  = sum_j E'_ij                (ACT accum_out, fused with the exp)
    PosE_i = sum_{t_all_j == t_i} E'_ij (one fused DVE scalar_tensor_tensor:
                                        (t_rep == t_i) * E', accum_out)
    S_i    = S_a_i + k1_i*PosE_i - e^{10 r2_i - 10}/n_i   (removes the j==i
             term and reweights positives from 1/cc to 1/(cc-1))
    numer_i/n_i = 10*(f_i.M[t_i] - r2_i)/n_i - 10
    mlp_i  = numer_i/n_i - log S_i
    loss   = -mean_i mlp_i
where n_c = bincount(targets), cc = n+1, r2_i = |f_i|^2 (computed from the
same fp16 values the PE sees so the diagonal cancels exactly), and
M[c] = sum of all A_j with class c.
"""

import sys
from contextlib import ExitStack

import numpy as np

sys.path.insert(0, "/opt/trn_rl_repo")

import concourse.bass as bass  # noqa: E402
import concourse.mybir as mybir  # noqa: E402
import concourse.tile as tile  # noqa: E402
from concourse import bacc  # noqa: E402
from concourse.bass_utils import run_bass_kernel_spmd  # noqa: E402

P = 128
TEMP = 0.1
SHIFT = 10.0
LB_PAD = -20.0  # pad column bias: exp(10*dot - 10 + 10*(-20)) == 0 in fp32

F16 = mybir.dt.float16
F32 = mybir.dt.float32
AF = mybir.ActivationFunctionType
ALU = mybir.AluOpType


def build_nc(n_rowtiles: int, n_chunks: int, chunk: int, iters: int = 1,
             stage: str = "full") -> bass.Bass:
    """One-core program; run SPMD on 8 cores with per-core inputs."""
    BL = n_rowtiles * P          # rows per core
    JP = n_chunks * chunk        # padded column count
    NSUB = chunk // 512
    assert chunk % 512 == 0

    # Bacc (not plain Bass): its compile() runs generate_event_semaphores(),
    # which splits multi-waits — walrus codegen allows 1 sync wait per inst.
    nc = bacc.Bacc(None)
    lhsT_d = nc.declare_dram_parameter("lhsT", [P, BL], F16, isOutput=False)
    fT_d = nc.declare_dram_parameter("fT", [P, JP], F16, isOutput=False)
    tR_d = nc.declare_dram_parameter("tR", [P, JP], F16, isOutput=False)
    lb_d = nc.declare_dram_parameter("lb", [1, JP], F16, isOutput=False)
    tpart_d = nc.declare_dram_parameter("tpart", [P, n_rowtiles], F16, isOutput=False)
    fnat_d = nc.declare_dram_parameter("fnat", [P, BL], F16, isOutput=False)
    mg_d = nc.declare_dram_parameter("mg", [P, BL], F16, isOutput=False)
    invn_d = nc.declare_dram_parameter("invn", [P, n_rowtiles], F32, isOutput=False)
    invn10_d = nc.declare_dram_parameter("invn10", [P, n_rowtiles], F32, isOutput=False)
    k1_d = nc.declare_dram_parameter("k1", [P, n_rowtiles], F32, isOutput=False)
    mlp_d = nc.declare_dram_parameter("mlp", [P, n_rowtiles], F32, isOutput=True)

    with tile.TileContext(nc) as tc, ExitStack() as ctx:
        const = ctx.enter_context(tc.tile_pool(name="const", bufs=1))
        epool = ctx.enter_context(tc.tile_pool(name="epool", bufs=3))
        jpool = ctx.enter_context(tc.tile_pool(name="jpool", bufs=2))
        psum = ctx.enter_context(
            tc.tile_pool(name="psum", bufs=2, space=bass.MemorySpace.PSUM)
        )

        for _it in range(iters):
            lhsT = const.tile([P, BL], F16)
            nc.sync.dma_start(lhsT[:], lhsT_d[:])
            ones = const.tile([1, P], F16)
            nc.vector.memset(ones[:], 1.0)
            lb = const.tile([1, JP], F16)
            nc.sync.dma_start(lb[:], lb_d[:])
            nbias = const.tile([P, 1], F32)
            nc.vector.memset(nbias[:], -SHIFT)
            zbias = const.tile([P, 1], F32)
            nc.vector.memset(zbias[:], 0.0)

            fTs, tRs = [], []
            for c in range(n_chunks):
                ft = const.tile([P, chunk], F16, tag=f"fT{c}")
                nc.sync.dma_start(ft[:], fT_d[:, c * chunk:(c + 1) * chunk])
                fTs.append(ft)
                tr = const.tile([P, chunk], F16, tag=f"tR{c}")
                nc.sync.dma_start(tr[:], tR_d[:, c * chunk:(c + 1) * chunk])
                tRs.append(tr)

            tpart = const.tile([P, n_rowtiles], F16)
            nc.sync.dma_start(tpart[:], tpart_d[:])
            fnat = const.tile([P, BL], F16)
            nc.sync.dma_start(fnat[:], fnat_d[:])
            mg = const.tile([P, BL], F16)
            nc.sync.dma_start(mg[:], mg_d[:])
            invn = const.tile([P, n_rowtiles], F32)
            nc.sync.dma_start(invn[:], invn_d[:])
            invn10 = const.tile([P, n_rowtiles], F32)
            nc.sync.dma_start(invn10[:], invn10_d[:])
            k1 = const.tile([P, n_rowtiles], F32)
            nc.sync.dma_start(k1[:], k1_d[:])

            sacc = const.tile([P, n_rowtiles * n_chunks], F32)
            pacc = const.tile([P, n_rowtiles * n_chunks], F32)

            for c in range(n_chunks if stage != "dma" else 0):
                for r in range(n_rowtiles):
                    pt = psum.tile([P, chunk], F32, tag="pt")
                    for s in range(NSUB):
                        sl = slice(s * 512, (s + 1) * 512)
                        nc.tensor.matmul(
                            pt[:, sl], lhsT[:, r * P:(r + 1) * P], fTs[c][:, sl],
                            start=True, stop=False,
                        )
                        nc.tensor.matmul(
                            pt[:, sl], ones[:, :],
                            lb[:, c * chunk + s * 512: c * chunk + (s + 1) * 512],
                            start=False, stop=True,
                        )
                    col = r * n_chunks + c
                    if stage == "mm":
                        nc.scalar.copy(sacc[:, col:col + 1], pt[:, 0:1])
                        continue
                    et = epool.tile([P, chunk], F16, tag="et")
                    nc.scalar.activation(
                        et[:], pt[:], AF.Exp, bias=nbias[:], scale=1.0 / TEMP,
                        accum_out=sacc[:, col:col + 1],
                    )
                    if stage == "act":
                        nc.vector.tensor_scalar_add(
                            pacc[:, col:col + 1], et[:, 0:1], 0.0)
                        continue
                    jt = jpool.tile([P, chunk], F16, tag="jt")
                    nc.vector.scalar_tensor_tensor(
                        out=jt[:], in0=tRs[c][:], scalar=tpart[:, r:r + 1], in1=et[:],
                        op0=ALU.is_equal, op1=ALU.mult,
                        accum_out=pacc[:, col:col + 1],
                    )

            if stage == "dma":
                nc.vector.memset(sacc[:], 1.0)
                nc.vector.memset(pacc[:], 1.0)
            # ---- epilogue: assemble per-row loss terms (tiny [P, n_rowtiles] ops)
            sa8 = const.tile([P, n_rowtiles], F32)
            pe8 = const.tile([P, n_rowtiles], F32)
            nc.vector.tensor_reduce(
                sa8[:], sacc[:].rearrange("p (r c) -> p r c", c=n_chunks),
                axis=mybir.AxisListType.X, op=ALU.add,
            )
            nc.vector.tensor_reduce(
                pe8[:], pacc[:].rearrange("p (r c) -> p r c", c=n_chunks),
                axis=mybir.AxisListType.X, op=ALU.add,
            )

            # row dots via scalar_tensor_tensor ((x*1)*y, fused row-sum);
            # tensor_tensor_reduce is avoided — it crashes the exec unit here.
            r2t = const.tile([P, n_rowtiles], F32)
            fmt = const.tile([P, n_rowtiles], F32)
            for r in range(n_rowtiles):
                rs = slice(r * P, (r + 1) * P)
                scr = jpool.tile([P, P], F32, tag="scr")
                nc.vector.scalar_tensor_tensor(
                    out=scr[:], in0=fnat[:, rs], scalar=1.0, in1=fnat[:, rs],
                    op0=ALU.mult, op1=ALU.mult,
                    accum_out=r2t[:, r:r + 1],
                )
                scr2 = jpool.tile([P, P], F32, tag="scr")
                nc.vector.scalar_tensor_tensor(
                    out=scr2[:], in0=fnat[:, rs], scalar=1.0, in1=mg[:, rs],
                    op0=ALU.mult, op1=ALU.mult,
                    accum_out=fmt[:, r:r + 1],
                )

            e1 = const.tile([P, n_rowtiles], F32)
            nc.scalar.activation(e1[:], r2t[:], AF.Exp, bias=nbias[:], scale=1.0 / TEMP)

            tA = const.tile([P, n_rowtiles], F32)
            nc.vector.tensor_tensor(tA[:], pe8[:], k1[:], ALU.mult)
            tB = const.tile([P, n_rowtiles], F32)
            nc.vector.tensor_tensor(tB[:], e1[:], invn[:], ALU.mult)
            tC = const.tile([P, n_rowtiles], F32)
            nc.vector.tensor_tensor(tC[:], tA[:], tB[:], ALU.subtract)
            St = const.tile([P, n_rowtiles], F32)
            nc.vector.tensor_tensor(St[:], tC[:], sa8[:], ALU.add)

            logS = const.tile([P, n_rowtiles], F32)
            nc.scalar.activation(logS[:], St[:], AF.Ln, bias=zbias[:], scale=1.0)

            y1 = const.tile([P, n_rowtiles], F32)
            nc.vector.tensor_tensor(y1[:], fmt[:], r2t[:], ALU.subtract)
            y2 = const.tile([P, n_rowtiles], F32)
            nc.vector.tensor_tensor(y2[:], y1[:], invn10[:], ALU.mult)
            z1 = const.tile([P, n_rowtiles], F32)
            nc.vector.tensor_tensor(z1[:], y2[:], logS[:], ALU.subtract)
            mlpt = const.tile([P, n_rowtiles], F32)
            nc.vector.tensor_scalar_add(mlpt[:], z1[:], -SHIFT)

            nc.sync.dma_start(mlp_d[:], mlpt[:])

    # Bacc defers register allocation and wait legalization to compile();
    # run_bass_kernel_spmd does not finalize a prebuilt module itself.
    nc.finalize()
    return nc


def prep_inputs(centers1, features, targets, n_cores, n_rowtiles, n_chunks, chunk):
    """Host-side sharding/layout prep. Returns per-core input maps."""
    B, D = features.shape
    C = centers1.shape[0]
    BL = n_rowtiles * P
    JP = n_chunks * chunk
    J = B + C
    assert BL * n_cores == B and D == P and JP >= J

    features = np.asarray(features, np.float32)
    centers1 = np.asarray(centers1, np.float32)
    targets = np.asarray(targets).astype(np.int64)

    n = np.bincount(targets, minlength=C).astype(np.int64)  # per-class counts
    cc = n + 1
    t_all = np.concatenate([targets, np.arange(C, dtype=np.int64)])

    # per-class fp16 bias value lb(c) = ln(1/cc_c)/10, and its exact effect
    lb_class16 = (np.log(1.0 / cc) / 10.0).astype(np.float16)
    atilde = np.exp(10.0 * lb_class16.astype(np.float64))  # realized a~_c

    lb_row = np.full((1, JP), LB_PAD, np.float16)
    lb_row[0, :J] = lb_class16[t_all]

    tR = np.full((JP,), -1.0, np.float16)
    tR[:J] = t_all.astype(np.float16)
    tR = np.ascontiguousarray(np.broadcast_to(tR, (P, JP)))

    feats_all = np.concatenate([features, centers1], axis=0)
    fT = np.zeros((P, JP), np.float16)
    fT[:, :J] = feats_all.T.astype(np.float16)

    # M[c] = sum of feature rows with target c, plus center c
    M = np.zeros((C, D), np.float64)
    np.add.at(M, targets, features.astype(np.float64))
    M += centers1
    Mg = M[targets].astype(np.float16)  # [B, D]

    n_t = n[targets].astype(np.float64)          # >= 1 for every row
    cc_t = cc[targets].astype(np.float64)
    k1_all = (1.0 / (n_t * cc_t * atilde[targets])).astype(np.float32)
    invn_all = (1.0 / n_t).astype(np.float32)
    invn10_all = (10.0 / n_t).astype(np.float32)

    def per_row_layout(x, dtype):
        # [BL(, D)] -> [P, n_rowtiles(*D)] with element (p, r(*D+d)) = row r*P+p
        x = x.reshape(n_rowtiles, P, -1).transpose(1, 0, 2)
        return np.ascontiguousarray(x.reshape(P, -1).astype(dtype))

    in_maps = []
    for k in range(n_cores):
        rows = slice(k * BL, (k + 1) * BL)
        in_maps.append({
            "lhsT": np.ascontiguousarray(fT[:, k * BL:(k + 1) * BL]),
            "fT": fT,
            "tR": tR,
            "lb": lb_row,
            "tpart": per_row_layout(targets[rows].astype(np.float16), np.float16),
            "fnat": per_row_layout(features[rows], np.float16),
            "mg": per_row_layout(Mg[rows], np.float16),
            "invn": per_row_layout(invn_all[rows], np.float32),
            "invn10": per_row_layout(invn10_all[rows], np.float32),
            "k1": per_row_layout(k1_all[rows], np.float32),
        })
    return in_maps


_NC_CACHE = {}


def _get_nc(n_rowtiles, n_chunks, chunk, iters=1, stage="full"):
    key = (n_rowtiles, n_chunks, chunk, iters, stage)
    if key not in _NC_CACHE:
        _NC_CACHE[key] = build_nc(n_rowtiles, n_chunks, chunk, iters, stage)
    return _NC_CACHE[key]


def run(centers1, features, targets, trace=False):
    n_cores, n_rowtiles, n_chunks, chunk = 8, 8, 6, 1536
    nc = _get_nc(n_rowtiles, n_chunks, chunk)
    in_maps = prep_inputs(
        centers1, features, targets, n_cores, n_rowtiles, n_chunks, chunk
    )
    res = run_bass_kernel_spmd(nc, in_maps, list(range(n_cores)), trace=trace)
    mlps = [res.results[k]["mlp"].T.reshape(-1) for k in range(n_cores)]
    loss = -np.mean(np.concatenate(mlps), dtype=np.float64)
    return np.float32(loss), res


def kernel(centers1, features, targets):
    loss, _ = run(centers1, features, targets)
    return np.asarray(loss, dtype=np.float32)



# revision 2
# speedup vs baseline: 1.6298x; 1.6298x over previous
"""Supervised-contrastive loss (balanced softmax variant) on 8 Trainium2 cores.

Transposed/class-sorted formulation. Rows are sorted by target class and
split 1024 per core; columns (all 8192 features + 1000 centers, merged and
class-sorted) are permuted per core so the core's "positive window" (all
columns whose class appears among its rows, <= 1280 of 9216) comes first.

Per j-tile (128 columns on partitions, 1024 rows on the free axis):
    PE : d[j, i] = A_j . f_i     fp8(e4m3) DoubleRow matmul (0.5 cyc/row)
    ACT: E'[j, i] = exp(10*d + (ln a_j - 10))   a_j = 1/cls_count (the
         per-COLUMN balanced-softmax weight rides in the per-PARTITION
         activation bias -- no bias matmul at all)
    DVE: Acc[t%2] += E'          fp16 tensor_tensor add (2x_1p mode)
    DVE (window tiles only): MAcc += (tcol_j == trow_i) * E'

The device returns the raw fp16 accumulators [128, 3*1024]; the host does
the cross-partition sums in f64 and finishes:
    S_i   = sum_j a_j E_ij        (from Acc)
    P_i   = sum_{cls_j == t_i} a_j E_ij   (from MAcc, includes self)
    S'_i  = S_i + (P_i - Eii)/n_i     n_i = bincount(targets)[t_i]
    mlp_i = 10*(f_i.M[t_i] - r2_i)/n_i - 10 - log(S'_i)
    loss  = -mean(mlp_i)
where Eii = exp(10*r28_i - 10) removes the self column exactly (r28 is the
self dot in the same fp8 values the PE saw), and M[c] is the class sum of
fp16 features + center.
"""

import sys
from contextlib import ExitStack

import numpy as np
import ml_dtypes

sys.path.insert(0, "/opt/trn_rl_repo")

import concourse.bass as bass  # noqa: E402
import concourse.mybir as mybir  # noqa: E402
import concourse.tile as tile  # noqa: E402
from concourse import bacc  # noqa: E402
from concourse.bass_utils import run_bass_kernel_spmd  # noqa: E402

P = 128
BL = 1024          # rows per core
NT = 72            # j-tiles of 128 columns (9216 padded)
JP = NT * P
MW = 10            # masked-window j-tiles (1280 cols >= max window)
TEMP = 0.1
SHIFT = 10.0

F8NP = ml_dtypes.float8_e4m3
F8 = mybir.dt.float8e4
F16 = mybir.dt.float16
F32 = mybir.dt.float32
AF = mybir.ActivationFunctionType
ALU = mybir.AluOpType


def build_nc() -> bass.Bass:
    """One-core program; run SPMD on 8 cores with per-core inputs."""
    nc = bacc.Bacc(None)
    featq_d = nc.declare_dram_parameter("featq", [64, 2 * BL], F8, isOutput=False)
    fTq_d = nc.declare_dram_parameter("fTq", [64, NT * 256], F8, isOutput=False)
    bias_d = nc.declare_dram_parameter("bias", [P, NT], F32, isOutput=False)
    tIrow_d = nc.declare_dram_parameter("tIrow", [P, BL], F16, isOutput=False)
    tcol_d = nc.declare_dram_parameter("tcol", [P, MW], F16, isOutput=False)
    accs_d = nc.declare_dram_parameter("accs", [P, 3 * BL], F16, isOutput=True)

    with tile.TileContext(nc) as tc, ExitStack() as ctx:
        const = ctx.enter_context(tc.tile_pool(name="const", bufs=1))
        epool = ctx.enter_context(tc.tile_pool(name="epool", bufs=3))
        mpool = ctx.enter_context(tc.tile_pool(name="mpool", bufs=2))
        psum = ctx.enter_context(
            tc.tile_pool(name="psum", bufs=2, space=bass.MemorySpace.PSUM)
        )

        featq = const.tile([64, 2 * BL], F8)
        nc.sync.dma_start(featq[:], featq_d[:])
        bias = const.tile([P, NT], F32)
        nc.sync.dma_start(bias[:], bias_d[:])
        tIrow = const.tile([P, BL], F16)
        nc.sync.dma_start(tIrow[:], tIrow_d[:])
        tcol = const.tile([P, MW], F16)
        nc.sync.dma_start(tcol[:], tcol_d[:])

        # weights stream, split so early tiles can start right away
        NCH = 8
        TPC = NT // NCH
        fTqs = []
        for c in range(NCH):
            ft = const.tile([64, TPC * 256], F8, tag=f"fTq{c}")
            nc.sync.dma_start(ft[:], fTq_d[:, c * TPC * 256:(c + 1) * TPC * 256])
            fTqs.append(ft)

        acc0 = const.tile([P, BL], F16)
        nc.vector.memset(acc0[:], 0.0)
        acc1 = const.tile([P, BL], F16)
        nc.vector.memset(acc1[:], 0.0)
        macc = const.tile([P, BL], F16)
        nc.vector.memset(macc[:], 0.0)
        accs = [acc0, acc1]

        for t in range(NT):
            lhs = fTqs[t // TPC][:, (t % TPC) * 256:(t % TPC + 1) * 256].rearrange(
                "p (k m) -> p k m", k=2
            )
            pt = psum.tile([P, BL], F32, tag="pt")
            for h in range(2):
                rhs = featq[:, h * BL:(h + 1) * BL].rearrange(
                    "p (k n) -> p k n", k=2
                )
                nc.tensor.matmul(
                    pt[:, h * 512:(h + 1) * 512], lhs, rhs,
                    start=True, stop=True,
                    perf_mode=mybir.MatmulPerfMode.DoubleRow,
                )
            et = epool.tile([P, BL], F16, tag="et")
            nc.scalar.activation(
                et[:], pt[:], AF.Exp, bias=bias[:, t:t + 1], scale=SHIFT,
            )
            a = accs[t % 2]
            nc.vector.tensor_tensor(a[:], a[:], et[:], ALU.add)
            if t < MW:
                mt = mpool.tile([P, BL], F16, tag="mt")
                nc.vector.scalar_tensor_tensor(
                    out=mt[:], in0=tIrow[:], scalar=tcol[:, t:t + 1], in1=et[:],
                    op0=ALU.is_equal, op1=ALU.mult,
                )
                nc.vector.tensor_tensor(macc[:], macc[:], mt[:], ALU.add)

        nc.sync.dma_start(accs_d[:, 0:BL], acc0[:])
        nc.sync.dma_start(accs_d[:, BL:2 * BL], acc1[:])
        nc.sync.dma_start(accs_d[:, 2 * BL:3 * BL], macc[:])

    nc.finalize()
    return nc


def prep_inputs(centers1, features, targets, n_cores):
    """Host-side sort/shard/layout prep. Returns per-core input maps and
    the per-core host epilogue data."""
    B, D = features.shape
    C = centers1.shape[0]
    J = B + C
    assert BL * n_cores == B and D == P and JP >= J

    feats16 = np.asarray(features, np.float32).astype(np.float16)
    cents16 = np.asarray(centers1, np.float32).astype(np.float16)
    targets = np.asarray(targets).astype(np.int64)

    n = np.bincount(targets, minlength=C).astype(np.int64)
    cc = n + 1

    order = np.argsort(targets, kind="stable")          # rows sorted by class
    # merged class-sorted columns: per class, feature rows then the center
    col_order = np.lexsort((
        np.concatenate([np.arange(B), np.full(C, B)]),
        np.concatenate([targets, np.arange(C)]),
    ))
    A16 = np.concatenate([feats16, cents16], axis=0)[col_order]   # [J, D]
    col_cls = np.concatenate([targets, np.arange(C)])[col_order]  # [J]
    a_col = 1.0 / cc[col_cls]

    # class sums for the numerator (f64 on the fp16 values)
    M = np.zeros((C, D))
    np.add.at(M, targets, feats16.astype(np.float64))
    M += cents16

    in_maps, host = [], []
    for k in range(n_cores):
        rids = order[k * BL:(k + 1) * BL]
        tcls = targets[rids]
        jlo = np.searchsorted(col_cls, tcls[0], "left")
        jhi = np.searchsorted(col_cls, tcls[-1], "right")
        assert jhi - jlo <= MW * P, f"core {k}: window {jhi - jlo} > {MW * P}"
        perm = np.concatenate([
            np.arange(jlo, jhi), np.arange(0, jlo), np.arange(jhi, J)
        ])

        Ap = np.zeros((JP, D), np.float16)
        Ap[:J] = A16[perm]
        up = np.full(JP, -1.0)
        up[:J] = col_cls[perm]
        biasv = np.full(JP, -200.0, np.float64)
        biasv[:J] = np.log(a_col[perm]) - SHIFT

        Aq8 = Ap.astype(F8NP)                     # [JP, 128]
        fq8 = feats16[rids].astype(F8NP)          # [BL, 128]

        # DoubleRow layouts: d = i*64 + p
        fTq = np.ascontiguousarray(
            Aq8.reshape(NT, P, 2, 64).transpose(3, 0, 2, 1).reshape(64, NT * 256)
        )
        featq = np.ascontiguousarray(
            fq8.reshape(2, 512, 2, 64).transpose(3, 0, 2, 1).reshape(64, 2 * BL)
        )

        in_maps.append({
            "featq": featq,
            "fTq": fTq,
            "bias": np.ascontiguousarray(biasv.reshape(NT, P).T.astype(np.float32)),
            "tIrow": np.ascontiguousarray(
                np.broadcast_to(tcls.astype(np.float16), (P, BL))
            ),
            "tcol": np.ascontiguousarray(
                up[:MW * P].reshape(MW, P).T.astype(np.float16)
            ),
        })

        n_t = n[tcls].astype(np.float64)
        r2_8 = np.einsum("ij,ij->i", fq8.astype(np.float64), fq8.astype(np.float64))
        Eii = np.exp(SHIFT * r2_8.astype(np.float32).astype(np.float64) - SHIFT)
        f16d = feats16[rids].astype(np.float64)
        r2n = np.einsum("ij,ij->i", f16d, f16d)
        numer = (SHIFT * (np.einsum("ij,ij->i", f16d, M[tcls]) - r2n)) / n_t - SHIFT
        host.append({"n_t": n_t, "Eii": Eii, "numer": numer})
    return in_maps, host


_NC_CACHE = {}


def _get_nc():
    if "nc" not in _NC_CACHE:
        _NC_CACHE["nc"] = build_nc()
    return _NC_CACHE["nc"]


def run(centers1, features, targets, trace=False):
    n_cores = 8
    nc = _get_nc()
    in_maps, host = prep_inputs(centers1, features, targets, n_cores)
    res = run_bass_kernel_spmd(nc, in_maps, list(range(n_cores)), trace=trace)
    mlps = []
    for k in range(n_cores):
        accs = res.results[k]["accs"].astype(np.float64)   # [128, 3*BL]
        S = accs[:, 0:BL].sum(0) + accs[:, BL:2 * BL].sum(0)
        Pm = accs[:, 2 * BL:3 * BL].sum(0)
        h = host[k]
        Sfix = S + (Pm - h["Eii"]) / h["n_t"]
        mlps.append(h["numer"] - np.log(Sfix))
    loss = -np.mean(np.concatenate(mlps))
    return np.float32(loss), res


def kernel(centers1, features, targets):
    loss, _ = run(centers1, features, targets)
    return np.asarray(loss, dtype=np.float32)


# revision 4
# speedup vs baseline: 1.6857x; 1.0343x over previous
"""Supervised-contrastive loss (balanced softmax variant) on 8 Trainium2 cores.

Transposed/class-sorted formulation. Rows are sorted by target class and
split 1024 per core; columns (all 8192 features + 1000 centers, merged and
class-sorted) are permuted per core so the core's "positive window" (all
columns whose class appears among its rows, <= 1280 of 9216) comes first.

Per j-tile (128 columns on partitions, 1024 rows on the free axis):
    PE : d[j, i] = A_j . f_i
    ACT: E'[j, i] = exp(10*d + (ln a_j - 10))   a_j = 1/cls_count (the
         per-COLUMN balanced-softmax weight rides in the per-PARTITION
         activation bias -- no bias matmul at all)
    DVE: Acc[t%2] += E'          fp16 tensor_tensor add (2x_1p mode)
    DVE (window tiles only): MAcc += (tcol_j == trow_i) * E'

The device returns the raw fp16 accumulators [128, 3*1024]; the host does
the cross-partition sums in f64 and finishes:
    S_i   = sum_j a_j E_ij        (from Acc)
    P_i   = sum_{cls_j == t_i} a_j E_ij   (from MAcc, includes self)
    S'_i  = S_i + (P_i - Eii)/n_i     n_i = bincount(targets)[t_i]
    mlp_i = 10*(f_i.M[t_i] - r2_i)/n_i - 10 - log(S'_i)
    loss  = -mean(mlp_i)
where Eii = exp(10*r2q_i - 10) removes the self column exactly (r2q is the
self dot in the same quantized values the PE saw), and M[c] is the class
sum of fp16 features + center.
"""

import sys
from contextlib import ExitStack

import numpy as np
import ml_dtypes

sys.path.insert(0, "/opt/trn_rl_repo")

import concourse.bass as bass  # noqa: E402
import concourse.mybir as mybir  # noqa: E402
import concourse.tile as tile  # noqa: E402
from concourse import bacc  # noqa: E402
from concourse.bass_utils import run_bass_kernel_spmd  # noqa: E402

P = 128
BL = 1024          # rows per core
NT = 72            # j-tiles of 128 columns (9216 padded)
JP = NT * P
MW = 10            # masked-window j-tiles (1280 cols >= max window)
TEMP = 0.1
SHIFT = 10.0

# --- experiment knobs ---
MM_FP8 = False     # fp8e4m3 DoubleRow matmul vs plain fp16
MM_SPLIT = 2       # matmuls per j-tile (N=512 is the ISA max per matmul)
PSUM_BUFS = 3
WCHUNK = 3         # j-tiles per fTq DMA chunk
OCHUNK = 4         # output DMA chunks per accumulator

F8NP = ml_dtypes.float8_e4m3
F8 = mybir.dt.float8e4
F16 = mybir.dt.float16
F32 = mybir.dt.float32
AF = mybir.ActivationFunctionType
ALU = mybir.AluOpType


def build_nc() -> bass.Bass:
    """One-core program; run SPMD on 8 cores with per-core inputs."""
    wpart = 64 if MM_FP8 else P
    wtile = 256 if MM_FP8 else P          # free elems per j-tile of weights
    nc = bacc.Bacc(None)
    featq_d = nc.declare_dram_parameter(
        "featq", [wpart, (2 * BL) if MM_FP8 else BL], F8 if MM_FP8 else F16,
        isOutput=False)
    fTq_d = nc.declare_dram_parameter(
        "fTq", [wpart, NT * wtile], F8 if MM_FP8 else F16, isOutput=False)
    bias_d = nc.declare_dram_parameter("bias", [P, NT], F32, isOutput=False)
    tIrow_d = nc.declare_dram_parameter("tIrow", [P, BL], F16, isOutput=False)
    tcol_d = nc.declare_dram_parameter("tcol", [P, MW], F16, isOutput=False)
    accs_d = nc.declare_dram_parameter("accs", [P, 3 * BL], F16, isOutput=True)

    with tile.TileContext(nc) as tc, ExitStack() as ctx:
        const = ctx.enter_context(tc.tile_pool(name="const", bufs=1))
        epool = ctx.enter_context(tc.tile_pool(name="epool", bufs=3))
        mpool = ctx.enter_context(tc.tile_pool(name="mpool", bufs=2))
        psum = ctx.enter_context(
            tc.tile_pool(name="psum", bufs=PSUM_BUFS, space=bass.MemorySpace.PSUM)
        )

        # warm the ACT exp table while DMAs are in flight
        warm = const.tile([P, 1], F32)
        nc.vector.memset(warm[:], 0.0)
        nc.scalar.activation(warm[:], warm[:], AF.Exp, bias=warm[:], scale=1.0)

        featq = const.tile([wpart, (2 * BL) if MM_FP8 else BL],
                           F8 if MM_FP8 else F16)
        half = featq.shape[1] // 2
        nc.sync.dma_start(featq[:, :half], featq_d[:, :half])
        nc.sync.dma_start(featq[:, half:], featq_d[:, half:])
        bias = const.tile([P, NT], F32)
        nc.sync.dma_start(bias[:], bias_d[:])
        tIrow = const.tile([P, BL], F16)
        nc.sync.dma_start(tIrow[:], tIrow_d[:])
        tcol = const.tile([P, MW], F16)
        nc.sync.dma_start(tcol[:], tcol_d[:])

        # weights stream, split so early tiles can start right away
        assert NT % WCHUNK == 0
        NCH = NT // WCHUNK
        fTqs = []
        for c in range(NCH):
            ft = const.tile([wpart, WCHUNK * wtile], F8 if MM_FP8 else F16,
                            tag=f"fTq{c}")
            nc.sync.dma_start(
                ft[:], fTq_d[:, c * WCHUNK * wtile:(c + 1) * WCHUNK * wtile])
            fTqs.append(ft)

        acc0 = const.tile([P, BL], F16)
        nc.vector.memset(acc0[:], 0.0)
        acc1 = const.tile([P, BL], F16)
        nc.vector.memset(acc1[:], 0.0)
        macc = const.tile([P, BL], F16)
        nc.vector.memset(macc[:], 0.0)
        accs = [acc0, acc1]

        for t in range(NT):
            lraw = fTqs[t // WCHUNK][:, (t % WCHUNK) * wtile:(t % WCHUNK + 1) * wtile]
            lhs = lraw.rearrange("p (k m) -> p k m", k=2) if MM_FP8 else lraw
            pt = psum.tile([P, BL], F32, tag="pt")
            NS = BL // MM_SPLIT
            for h in range(MM_SPLIT):
                if MM_FP8:
                    rhs = featq[:, h * NS * 2:(h + 1) * NS * 2].rearrange(
                        "p (k n) -> p k n", k=2)
                else:
                    rhs = featq[:, h * NS:(h + 1) * NS]
                nc.tensor.matmul(
                    pt[:, h * NS:(h + 1) * NS], lhs, rhs,
                    start=True, stop=True,
                    perf_mode=mybir.MatmulPerfMode.DoubleRow if MM_FP8 else None,
                )
            et = epool.tile([P, BL], F16, tag="et")
            nc.scalar.activation(
                et[:], pt[:], AF.Exp, bias=bias[:, t:t + 1], scale=SHIFT,
            )
            a = accs[t % 2]
            nc.vector.tensor_tensor(a[:], a[:], et[:], ALU.add)
            if t < MW:
                mt = mpool.tile([P, BL], F16, tag="mt")
                nc.vector.scalar_tensor_tensor(
                    out=mt[:], in0=tIrow[:], scalar=tcol[:, t:t + 1], in1=et[:],
                    op0=ALU.is_equal, op1=ALU.mult,
                )
                nc.vector.tensor_tensor(macc[:], macc[:], mt[:], ALU.add)
            if t == MW - 1:
                for o in range(OCHUNK):
                    sl = slice(o * BL // OCHUNK, (o + 1) * BL // OCHUNK)
                    nc.sync.dma_start(accs_d[:, 2 * BL:3 * BL][:, sl], macc[:, sl])

        for src, base in ((acc0, 0), (acc1, BL)):
            for o in range(OCHUNK):
                sl = slice(o * BL // OCHUNK, (o + 1) * BL // OCHUNK)
                nc.sync.dma_start(accs_d[:, base:base + BL][:, sl], src[:, sl])

    nc.finalize()
    return nc


def prep_inputs(centers1, features, targets, n_cores):
    """Host-side sort/shard/layout prep. Returns per-core input maps and
    the per-core host epilogue data."""
    B, D = features.shape
    C = centers1.shape[0]
    J = B + C
    assert BL * n_cores == B and D == P and JP >= J

    feats16 = np.asarray(features, np.float32).astype(np.float16)
    cents16 = np.asarray(centers1, np.float32).astype(np.float16)
    targets = np.asarray(targets).astype(np.int64)

    n = np.bincount(targets, minlength=C).astype(np.int64)
    cc = n + 1

    order = np.argsort(targets, kind="stable")          # rows sorted by class
    # merged class-sorted columns: per class, feature rows then the center
    col_order = np.lexsort((
        np.concatenate([np.arange(B), np.full(C, B)]),
        np.concatenate([targets, np.arange(C)]),
    ))
    A16 = np.concatenate([feats16, cents16], axis=0)[col_order]   # [J, D]
    col_cls = np.concatenate([targets, np.arange(C)])[col_order]  # [J]
    a_col = 1.0 / cc[col_cls]

    # class sums for the numerator (f64 on the fp16 values)
    M = np.zeros((C, D))
    np.add.at(M, targets, feats16.astype(np.float64))
    M += cents16

    in_maps, host = [], []
    for k in range(n_cores):
        rids = order[k * BL:(k + 1) * BL]
        tcls = targets[rids]
        jlo = np.searchsorted(col_cls, tcls[0], "left")
        jhi = np.searchsorted(col_cls, tcls[-1], "right")
        assert jhi - jlo <= MW * P, f"core {k}: window {jhi - jlo} > {MW * P}"
        perm = np.concatenate([
            np.arange(jlo, jhi), np.arange(0, jlo), np.arange(jhi, J)
        ])

        Ap = np.zeros((JP, D), np.float16)
        Ap[:J] = A16[perm]
        up = np.full(JP, -1.0)
        up[:J] = col_cls[perm]
        biasv = np.full(JP, -200.0, np.float64)
        biasv[:J] = np.log(a_col[perm]) - SHIFT

        if MM_FP8:
            Aq = Ap.astype(F8NP)                     # [JP, 128]
            fq = feats16[rids].astype(F8NP)          # [BL, 128]
            # DoubleRow layouts: d = i*64 + p
            fTq = np.ascontiguousarray(
                Aq.reshape(NT, P, 2, 64).transpose(3, 0, 2, 1).reshape(64, NT * 256)
            )
            featq = np.ascontiguousarray(
                fq.reshape(2, 512, 2, 64).transpose(3, 0, 2, 1).reshape(64, 2 * BL)
            )
        else:
            Aq = Ap
            fq = feats16[rids]
            fTq = np.ascontiguousarray(Aq.reshape(NT, P, P).transpose(2, 0, 1)
                                       .reshape(P, NT * P))
            featq = np.ascontiguousarray(fq.T)

        in_maps.append({
            "featq": featq,
            "fTq": fTq,
            "bias": np.ascontiguousarray(biasv.reshape(NT, P).T.astype(np.float32)),
            "tIrow": np.ascontiguousarray(
                np.broadcast_to(tcls.astype(np.float16), (P, BL))
            ),
            "tcol": np.ascontiguousarray(
                up[:MW * P].reshape(MW, P).T.astype(np.float16)
            ),
        })

        n_t = n[tcls].astype(np.float64)
        fqd = fq.astype(np.float64)
        r2q = np.einsum("ij,ij->i", fqd, fqd)
        Eii = np.exp(SHIFT * r2q.astype(np.float32).astype(np.float64) - SHIFT)
        f16d = feats16[rids].astype(np.float64)
        r2n = np.einsum("ij,ij->i", f16d, f16d)
        numer = (SHIFT * (np.einsum("ij,ij->i", f16d, M[tcls]) - r2n)) / n_t - SHIFT
        host.append({"n_t": n_t, "Eii": Eii, "numer": numer})
    return in_maps, host


_NC_CACHE = {}


def _get_nc():
    if "nc" not in _NC_CACHE:
        _NC_CACHE["nc"] = build_nc()
    return _NC_CACHE["nc"]


def run(centers1, features, targets, trace=False):
    n_cores = 8
    nc = _get_nc()
    in_maps, host = prep_inputs(centers1, features, targets, n_cores)
    res = run_bass_kernel_spmd(nc, in_maps, list(range(n_cores)), trace=trace)
    mlps = []
    for k in range(n_cores):
        accs = res.results[k]["accs"].astype(np.float64)   # [128, 3*BL]
        S = accs[:, 0:BL].sum(0) + accs[:, BL:2 * BL].sum(0)
        Pm = accs[:, 2 * BL:3 * BL].sum(0)
        h = host[k]
        Sfix = S + (Pm - h["Eii"]) / h["n_t"]
        mlps.append(h["numer"] - np.log(Sfix))
    loss = -np.mean(np.concatenate(mlps))
    return np.float32(loss), res


def kernel(centers1, features, targets):
    loss, _ = run(centers1, features, targets)
    return np.asarray(loss, dtype=np.float32)
